# revision 2
# baseline (speedup 1.0000x reference)
"""LinearAttention Trainium2 kernel v2: data-parallel over batch on 8 cores.

Reference per batch b (C=256 channels, L=4096 seq, H=8 heads, D=64):
  qkv = w_qkv @ x[b]; q,k,v = split(qkv)        # each (512, L)
  k = softmax(k, axis=L)
  ctx[h] = k[h] @ v[h].T                        # (64, 64)
  y[b] = w_out @ concat_h(ctx[h].T @ q[h]) + b_out

Key identity: everything downstream of ctx is linear in x, so
  y[b] = M x[b] + b_out,   M = sum_h Wout_h ctx_h^T Wq_h   (256 x 256).
This removes the Q projection and the attention-out GEMM entirely; the
final projection is a single (256x256) @ (256xL) GEMM.

Per-core design (2 batches/core):
  - K^T, V^T computed with L on partitions (lhsT = x chunk, rhs = w^T).
  - per-head transposed context with a ones-augmented v (65-wide lhsT):
    the extra output row (psum partition 64) is sum_l expk = the softmax
    denominator, for free - no separate ones-row matmul.
  - den row -> 4 PE transposes -> reciprocal -> per-partition ACT scale.
  - fold chain (all tiny): McT[d,o] = sum_e ctxT[e,d] wo[e,o] (scaled 1/den),
    Mt[c,o] = sum_d wq[d,c] McT[d,o];  y = Mt^T x + b.
  - all TensorE compute in bf16 (f32 PSUM accumulation), bf16 output.
  Scheduling (all engines are in-order queues):
  - ctx(lp-1) is emitted after proj(lp) so the PE never waits on the
    exp/v-copy of the chunk it is about to contract.
  - batch bi's fold chain is interleaved into batch bi+1's phase-1 (and
    b1's fold into b0's phase-3) so its DVE->PE->ACT->PE->Pool serial
    latency hides under dense matmul work.
  - phase-3 bias-adds alternate ACT/DVE so the psum consumer keeps up
    with the PE producer.
"""

import numpy as np

B, C, L = 16, 256, 4096
HID = 512
HEADS = 8
N_CORES = 8
NB = B // N_CORES  # batches per core
CC = C // 128  # contraction chunks for the input projections (2)
LP = L // 128  # l-tiles with l on partitions (32)
LT = L // 512  # l-tiles of 512 for moving-dim matmuls (8)
PR = HID // 128  # d-chunks of 128 (4): each = 2 heads of 64

_CACHE = {}


def _build(reps=1):
    from concourse import bacc, mybir, tile
    import concourse.bass as bass

    bf16 = mybir.dt.bfloat16
    f32 = mybir.dt.float32
    Exp = mybir.ActivationFunctionType.Exp
    Copy = mybir.ActivationFunctionType.Copy
    Ident = mybir.ActivationFunctionType.Identity

    nc = bacc.Bacc(
        "TRN2",
        target_bir_lowering=False,
        debug=False,
        enable_asserts=False,
        num_devices=N_CORES,
    )

    x_d = nc.dram_tensor("x", [NB, CC, 128, L], bf16, kind="ExternalInput")
    wk_d = nc.dram_tensor("wk_t", [CC, 128, HID], bf16, kind="ExternalInput")
    wv_d = nc.dram_tensor("wv_t", [CC, 128, HID], bf16, kind="ExternalInput")
    wq_d = nc.dram_tensor("wq_n", [PR, 128, C], bf16, kind="ExternalInput")
    wo_d = nc.dram_tensor("wo_h", [64, HEADS, C], bf16, kind="ExternalInput")
    bb_d = nc.dram_tensor("bb", [128, 2], f32, kind="ExternalInput")
    out_d = nc.dram_tensor("out", [NB, 2, 128, L], bf16, kind="ExternalOutput")

    with tile.TileContext(nc) as tc:
        with (
            tc.tile_pool(name="const", bufs=1) as const,
            tc.tile_pool(name="xp", bufs=4) as xp,
            tc.tile_pool(name="big", bufs=1) as big,
            tc.tile_pool(name="small", bufs=2) as small,
            tc.tile_pool(name="ostp", bufs=4) as ostp,
            tc.tile_pool(name="ps_mm", bufs=5, space="PSUM") as ps_mm,
            tc.tile_pool(name="ps_ctx", bufs=2, space="PSUM") as ps_ctx,
            tc.tile_pool(name="ps_small", bufs=1, space="PSUM") as ps_small,
        ):
            wk = const.tile([128, CC, HID], bf16)
            wv = const.tile([128, CC, HID], bf16)
            wqn = const.tile([128, PR, C], bf16)
            wo2 = const.tile([64, HEADS, C], bf16)
            bb = const.tile([128, 2], f32)
            id65 = const.tile([65, 1], f32)

            for cc in range(CC):
                nc.sync.dma_start(wk[:, cc, :], wk_d[cc])
                nc.sync.dma_start(wv[:, cc, :], wv_d[cc])
            for pr in range(PR):
                nc.sync.dma_start(wqn[:, pr, :], wq_d[pr])
            nc.sync.dma_start(wo2[:], wo_d[:])
            nc.sync.dma_start(bb[:], bb_d[:])
            nc.gpsimd.memset(id65[:], 1.0)

            # v with a ones column appended per head: [128, lp, h, 65].
            # memset once to 1.0 - data columns are overwritten every use,
            # the 65th columns stay 1 forever (bufs=1, const-like).
            vta = const.tile([128, LP, HEADS, 65], bf16)
            nc.gpsimd.memset(vta[:], 1.0)

            state = {}  # per-batch live tiles: xt, expkt, ctx_big, mt
            uid = [0]
            prefetched = {}  # (rep, bi) -> xt tile already being DMA-loaded

            def load_x(rep, bi):
                uid[0] += 1
                xt = xp.tile([128, CC, L], bf16, tag="xt", name=f"xt_{uid[0]}")
                # Pool-engine (SWDGE) ring: keeps the x prefetch off the SP
                # ring where phase-3 out-writes would head-of-line block it.
                for cc in range(CC):
                    nc.gpsimd.dma_start(xt[:, cc, :], x_d[bi, cc])
                prefetched[(rep, bi)] = xt

            def phase1(rep, bi, inject):
                """K^T/V^T projections fused with per-head ones-augmented
                context accumulation; `inject` pieces are drained at fixed
                lp points."""
                if (rep, bi) not in prefetched:
                    load_x(rep, bi)
                st = state[bi] = {}
                xt = st["xt"] = prefetched.pop((rep, bi))
                expkt = st["expkt"] = big.tile([128, LP, HID], bf16, tag=f"expkt{bi}", name=f"expkt_{uid[0]}")
                ctx_big = st["ctx"] = ps_ctx.tile([65, HEADS, 64], f32, tag="ctx", name=f"ctx_{uid[0]}")

                def proj(lp):
                    psk = ps_mm.tile([128, HID], f32, tag="mm")
                    psv = ps_mm.tile([128, HEADS, 64], f32, tag="mm")
                    for cc in range(CC):
                        nc.tensor.matmul(
                            psk[:],
                            xt[:, cc, lp * 128 : (lp + 1) * 128],
                            wk[:, cc, :],
                            start=(cc == 0),
                            stop=(cc == CC - 1),
                        )
                    for cc in range(CC):
                        nc.tensor.matmul(
                            psv[:].rearrange("p h e -> p (h e)"),
                            xt[:, cc, lp * 128 : (lp + 1) * 128],
                            wv[:, cc, :],
                            start=(cc == 0),
                            stop=(cc == CC - 1),
                        )
                    nc.scalar.activation(expkt[:, lp, :], psk[:], Exp)
                    # strided store: v head-chunks land at columns 0..63 of
                    # each 65-wide group, ones column untouched.
                    nc.vector.tensor_copy(vta[:, lp, :, 0:64], psv[:])

                def ctx(lp):
                    for h in range(HEADS):
                        # start=True lazily zeroes the ENTIRE 2KB psum bank
                        # row (ZERO_REGION_SIZE), so only the first matmul
                        # into the bank may carry it; the other heads' first
                        # writes inherit the pending-zero.
                        nc.tensor.matmul(
                            ctx_big[:, h, :],
                            vta[:, lp, h, :],
                            expkt[:, lp, h * 64 : (h + 1) * 64],
                            start=(lp == 0 and h == 0),
                            stop=(lp == LP - 1),
                            skip_group_check=True,
                        )

                proj(0)
                for lp in range(1, LP):
                    proj(lp)
                    ctx(lp - 1)
                    if lp % 6 == 0:
                        next(inject, None)
                ctx(LP - 1)

            def fold(bi):
                """Generator: ctx -> M fold chain in 4 pieces (tiny ops on
                DVE/PE/ACT/Pool), meant to be interleaved into dense work."""
                st = state[bi]
                ctx_big = st["ctx"]
                # per-head ctxT (bf16) + denominator row -> sbuf.
                ctxs = small.tile([64, HEADS, 64], bf16, tag="ctxs")
                nc.vector.tensor_copy(ctxs[:], ctx_big[0:64, :, :])
                densb = small.tile([65, HID], f32, tag="densb")
                nc.vector.tensor_copy(
                    densb[64:65, :],
                    ctx_big[64:65, :, :].rearrange("p h e -> p (h e)"),
                )
                yield
                tps = ps_small.tile([128, PR], f32, tag="tps")
                for j in range(PR):
                    nc.tensor.transpose(
                        tps[:, j : j + 1],
                        densb[64:65, j * 128 : (j + 1) * 128],
                        id65[64:65, :],
                    )
                inv_den = small.tile([128, PR], f32, tag="invden")
                nc.vector.reciprocal(inv_den[:], tps[:])
                yield
                # McT[d, o] = sum_e ctxT[e, d] wo[e, o], scaled by 1/den[d].
                mct = small.tile([128, PR, C], bf16, tag="mct")
                for pr in range(PR):
                    mc_ps = ps_mm.tile([128, C], f32, tag="mm")
                    for t in range(2):
                        h = 2 * pr + t
                        nc.tensor.matmul(
                            mc_ps[t * 64 : t * 64 + 64, :],
                            ctxs[:, h, :],
                            wo2[:, h, :],
                            start=True,
                            stop=True,
                            skip_group_check=True,
                        )
                    nc.scalar.activation(
                        mct[:, pr, :], mc_ps[:], Copy, scale=inv_den[:, pr : pr + 1]
                    )
                yield
                # Mt[c, o] = sum_d wq[d, c] McT[d, o]  (contract all 512 d).
                uid[0] += 1
                mt = st["mt"] = small.tile([128, CC, C], bf16, tag="mt", name=f"mt_{uid[0]}")
                for ch in range(CC):
                    mt_ps = ps_mm.tile([128, C], f32, tag="mm")
                    for pr in range(PR):
                        nc.tensor.matmul(
                            mt_ps[:],
                            wqn[:, pr, ch * 128 : (ch + 1) * 128],
                            mct[:, pr, :],
                            start=(pr == 0),
                            stop=(pr == PR - 1),
                        )
                    nc.scalar.activation(mt[:, ch, :], mt_ps[:], Copy)
                yield

            def loader(rep, bi):
                """Injectable one-shot x prefetch piece: the DMA is ordered
                behind the PE progress at its emission point, so it must be
                emitted a full phase before the data is needed."""
                if rep < reps:
                    load_x(rep, bi)
                yield

            def chain(*gens):
                for g in gens:
                    yield from g

            def phase3(rep, bi, inject):
                """y = Mt^T x + b per l-chunk of 512; bias-add alternates
                ACT/DVE so the psum consumer keeps pace with the PE."""
                st = state[bi]
                xt, mt = st["xt"], st["mt"]
                for lt in range(LT):
                    ostg = ostp.tile([128, 2, 512], bf16, tag="ostg")
                    for oc2 in range(2):
                        psf = ps_mm.tile([128, 512], f32, tag="mm")
                        for cc in range(CC):
                            nc.tensor.matmul(
                                psf[:],
                                mt[:, cc, oc2 * 128 : (oc2 + 1) * 128],
                                xt[:, cc, lt * 512 : (lt + 1) * 512],
                                start=(cc == 0),
                                stop=(cc == CC - 1),
                            )
                        if oc2 == 0:
                            nc.scalar.activation(
                                ostg[:, oc2, :],
                                psf[:],
                                Ident,
                                bias=bb[:, oc2 : oc2 + 1],
                            )
                        else:
                            nc.vector.tensor_scalar_add(
                                ostg[:, oc2, :], psf[:], bb[:, oc2 : oc2 + 1]
                            )
                        nc.sync.dma_start(
                            out_d[bi, oc2, :, lt * 512 : (lt + 1) * 512],
                            ostg[:, oc2, :],
                        )
                    next(inject, None)

            for rep in range(reps):
                phase1(rep, 0, iter(()))
                phase1(rep, 1, chain(loader(rep + 1, 0), fold(0)))
                phase3(rep, 0, chain(loader(rep + 1, 1), fold(1)))
                phase3(rep, 1, iter(()))

    nc.compile()
    return nc


def _get_nc():
    if "nc" not in _CACHE:
        _CACHE["nc"] = _build()
    return _CACHE["nc"]


def _prep_in_maps(x, w_qkv, w_out, b_out):
    import ml_dtypes

    bf16 = ml_dtypes.bfloat16
    wq_n = np.ascontiguousarray(w_qkv[0:512].reshape(PR, 128, C)).astype(bf16)
    wk_t = np.ascontiguousarray(w_qkv[512:1024].T).reshape(CC, 128, HID).astype(bf16)
    wv_t = np.ascontiguousarray(w_qkv[1024:1536].T).reshape(CC, 128, HID).astype(bf16)
    # wo_h[e_local, h, o] = w_out[o, h*64 + e_local]
    wo_h = np.ascontiguousarray(
        w_out.T.reshape(HEADS, 64, C).transpose(1, 0, 2)
    ).astype(bf16)
    bb = np.ascontiguousarray(b_out.reshape(2, 128).T).astype(np.float32)
    in_maps = []
    for c in range(N_CORES):
        xs = x[c * NB : (c + 1) * NB].reshape(NB, CC, 128, L).astype(bf16)
        in_maps.append(
            {
                "x": np.ascontiguousarray(xs),
                "wq_n": wq_n,
                "wk_t": wk_t,
                "wv_t": wv_t,
                "wo_h": wo_h,
                "bb": bb,
            }
        )
    return in_maps


def kernel(x, w_qkv, w_out, b_out):
    from concourse.bass_utils import run_bass_kernel_spmd

    nc = _get_nc()
    in_maps = _prep_in_maps(
        np.asarray(x, dtype=np.float32),
        np.asarray(w_qkv, dtype=np.float32),
        np.asarray(w_out, dtype=np.float32),
        np.asarray(b_out, dtype=np.float32),
    )
    res = run_bass_kernel_spmd(nc, in_maps, core_ids=list(range(N_CORES)))
    out = np.concatenate(
        [
            res.results[c]["out"].astype(np.float32).reshape(NB, C, L)
            for c in range(N_CORES)
        ],
        axis=0,
    )
    return out.astype(np.float32)


# revision 5
# speedup vs baseline: 1.0976x; 1.0976x over previous
"""LinearAttention TRN2 kernel v3: v2 + W_v fold via DMA-transposed x.

v3 on top of v2's M-fold (y = M x + b): the V projection disappears too.
  ctx_h^T[e,d] = sum_c WvT[c,e] G[c,d],  G[c,d] = sum_l x[c,l] expk[d,l].
G is accumulated on the PE from XBAR-DMA-transposed x chunks (lhsT) against
expk (rhs) - same cycle count as the old V projection, but the separate
per-head context accumulation (16.4k cyc/batch) collapses into a tiny fold,
and the softmax denominator comes from 1-moving matmuls (lhsT=expk chunk,
rhs=ones) accumulating directly in d-on-partitions orientation - no
denominator transposes. Net: ~15k PE cycles/batch less than v2.

Scheduling is v2's: in-order queues, G/den pipelined two chunks behind the
K projection, fold chains interleaved into the next batch's dense phase
(piece 1 - the G/den psum evacuation - drains at phase start so the single
set of G banks can be reused), x and xT prefetched a phase early on the
Pool/DVE DMA rings, phase-3 bias-adds alternate ACT/DVE, bf16 output.
"""

import numpy as np

B, C, L = 16, 256, 4096
HID = 512
HEADS = 8
N_CORES = 8
NB = B // N_CORES  # batches per core
CC = C // 128  # contraction chunks for the input projections (2)
LP = L // 128  # l-tiles with l on partitions (32)
LT = L // 512  # l-tiles of 512 for moving-dim matmuls (8)
PR = HID // 128  # d-chunks of 128 (4): each = 2 heads of 64

_CACHE = {}


def _build(reps=1):
    from concourse import bacc, mybir, tile
    import concourse.bass as bass

    bf16 = mybir.dt.bfloat16
    f32 = mybir.dt.float32
    Exp = mybir.ActivationFunctionType.Exp
    Copy = mybir.ActivationFunctionType.Copy
    Ident = mybir.ActivationFunctionType.Identity

    nc = bacc.Bacc(
        "TRN2",
        target_bir_lowering=False,
        debug=False,
        enable_asserts=False,
        num_devices=N_CORES,
    )

    x_d = nc.dram_tensor("x", [NB, CC, 128, L], bf16, kind="ExternalInput")
    wk_d = nc.dram_tensor("wk_t", [CC, 128, HID], bf16, kind="ExternalInput")
    wv_d = nc.dram_tensor("wv_t", [CC, 128, HID], bf16, kind="ExternalInput")
    wq_d = nc.dram_tensor("wq_n", [PR, 128, C], bf16, kind="ExternalInput")
    wo_d = nc.dram_tensor("wo_h", [64, HEADS, C], bf16, kind="ExternalInput")
    bb_d = nc.dram_tensor("bb", [128, 2], f32, kind="ExternalInput")
    out_d = nc.dram_tensor("out", [NB, 2, 128, L], bf16, kind="ExternalOutput")

    with tile.TileContext(nc) as tc:
        with (
            tc.tile_pool(name="const", bufs=1) as const,
            tc.tile_pool(name="xp", bufs=4) as xp,
            tc.tile_pool(name="xtp", bufs=3) as xtp,
            tc.tile_pool(name="ekp", bufs=4) as ekp,
            tc.tile_pool(name="small", bufs=2) as small,
            tc.tile_pool(name="ostp", bufs=4) as ostp,
            tc.tile_pool(name="ps_mm", bufs=5, space="PSUM") as ps_mm,
            tc.tile_pool(name="ps_g", bufs=2, space="PSUM") as ps_g,
            tc.tile_pool(name="ps_den", bufs=1, space="PSUM") as ps_den,
        ):
            wk = const.tile([128, CC, HID], bf16)
            wv = const.tile([128, CC, HID], bf16)
            wqn = const.tile([128, PR, C], bf16)
            wo2 = const.tile([64, HEADS, C], bf16)
            bb = const.tile([128, 2], f32)
            ones_col = const.tile([128, 1], bf16)

            for cc in range(CC):
                nc.sync.dma_start(wk[:, cc, :], wk_d[cc])
                nc.sync.dma_start(wv[:, cc, :], wv_d[cc])
            for pr in range(PR):
                nc.sync.dma_start(wqn[:, pr, :], wq_d[pr])
            nc.sync.dma_start(wo2[:], wo_d[:])
            nc.sync.dma_start(bb[:], bb_d[:])
            nc.gpsimd.memset(ones_col[:], 1.0)

            state = {}
            uid = [0]
            prefetched = {}  # (rep, bi) -> (xt, xts) tiles being DMA-loaded

            def load_x(rep, bi):
                uid[0] += 1
                xt = xp.tile([128, CC, L], bf16, tag="xt", name=f"xt_{uid[0]}")
                xts = xtp.tile(
                    [128, LP, CC, 128], bf16, tag="xts", name=f"xts_{uid[0]}"
                )
                # x on the Pool SWDGE ring, transposed x via the ACT HWDGE
                # ring's XBAR: out[p, lp, c] = x[c, lp*128 + p].
                for cc in range(CC):
                    nc.gpsimd.dma_start(xt[:, cc, :], x_d[bi, cc])
                    nc.sync.dma_start_transpose(xts[:, :, cc, :], x_d[bi, cc])
                prefetched[(rep, bi)] = (xt, xts)

            def phase1(rep, bi, inject):
                """K projection + exp fused with G/den accumulation, G/den
                pipelined two chunks behind; `inject` drains once at start
                (previous batch's psum evacuation) then at fixed lp points."""
                if (rep, bi) not in prefetched:
                    load_x(rep, bi)
                st = state[bi] = {}
                xt, xts = prefetched.pop((rep, bi))
                st["xt"] = xt
                next(inject, None)  # prev batch's G/den evacuation
                g_ps = [
                    ps_g.tile([128, HID], f32, tag="g", name=f"g_{uid[0]}_{cc}")
                    for cc in range(CC)
                ]
                den_ps = ps_den.tile([128, PR], f32, tag="den", name=f"den_{uid[0]}")
                st["g"] = g_ps
                st["den"] = den_ps
                eks = {}

                def proj(lp):
                    psk = ps_mm.tile([128, HID], f32, tag="mm")
                    for cc in range(CC):
                        nc.tensor.matmul(
                            psk[:],
                            xt[:, cc, lp * 128 : (lp + 1) * 128],
                            wk[:, cc, :],
                            start=(cc == 0),
                            stop=(cc == CC - 1),
                        )
                    ek = ekp.tile([128, HID], bf16, tag="ek")
                    nc.scalar.activation(ek[:], psk[:], Exp)
                    eks[lp] = ek

                def gden(lp):
                    ek = eks.pop(lp)
                    # G accumulation; den's tiny 1-moving matmuls interleave
                    # between the 512-moving ones so their stationary loads
                    # hide under execution.
                    for cc in range(CC):
                        nc.tensor.matmul(
                            g_ps[cc][:],
                            xts[:, lp, cc, :],
                            ek[:],
                            start=(lp == 0),
                            stop=(lp == LP - 1),
                        )
                        for pr in (cc, cc + 2):
                            # single bank: only the very first write starts
                            # (whole-bank lazy zero), others inherit it.
                            nc.tensor.matmul(
                                den_ps[:, pr : pr + 1],
                                ek[:, pr * 128 : (pr + 1) * 128],
                                ones_col[:],
                                start=(lp == 0 and pr == 0),
                                stop=(lp == LP - 1),
                                skip_group_check=True,
                            )

                proj(0)
                proj(1)
                for lp in range(2, LP):
                    proj(lp)
                    gden(lp - 2)
                    if lp % 6 == 0:
                        next(inject, None)
                gden(LP - 2)
                gden(LP - 1)

            def fold(bi, mid=None):
                """Generator: G/den -> M fold chain in 4 pieces; `mid()`
                (the next batch's x/xT prefetch) is emitted with piece 2 so
                the transfers get a full phase of lead time."""
                st = state[bi]
                g_ps, den_ps = st["g"], st["den"]
                # piece 1: evacuate psum (frees G/den banks for next batch).
                gs = small.tile([128, CC, HID], bf16, tag="gs")
                for cc in range(CC):
                    nc.vector.tensor_copy(gs[:, cc, :], g_ps[cc][:])
                inv_den = small.tile([128, PR], f32, tag="invden")
                nc.vector.reciprocal(inv_den[:], den_ps[:])
                yield
                if mid is not None:
                    mid()
                # piece 2: per-head ctxT[e,d] = sum_c WvT[c,e] G[c,d].
                ctx_ps = ps_mm.tile([64, HEADS, 64], f32, tag="mm")
                for h in range(HEADS):
                    for cc in range(CC):
                        nc.tensor.matmul(
                            ctx_ps[:, h, :],
                            wv[:, cc, h * 64 : (h + 1) * 64],
                            gs[:, cc, h * 64 : (h + 1) * 64],
                            start=(h == 0 and cc == 0),
                            stop=(cc == CC - 1),
                            skip_group_check=True,
                        )
                ctxs = small.tile([64, HEADS, 64], bf16, tag="ctxs")
                nc.vector.tensor_copy(ctxs[:], ctx_ps[:])
                yield
                # piece 3: McT[d,o] = sum_e ctxT[e,d] wo[e,o], scaled 1/den.
                # Two pr column-blocks share one psum bank to keep the ps_mm
                # rotation deep. Pending-zero marking is PER-PARTITION: the
                # first write of EACH partition range must carry start.
                mct = small.tile([128, PR, C], bf16, tag="mct")
                for j in range(PR // 2):
                    mc_ps = ps_mm.tile([128, 2, C], f32, tag="mm")
                    for u in range(2):
                        pr = 2 * j + u
                        for t in range(2):
                            h = 2 * pr + t
                            nc.tensor.matmul(
                                mc_ps[t * 64 : t * 64 + 64, u, :],
                                ctxs[:, h, :],
                                wo2[:, h, :],
                                start=(u == 0),
                                stop=True,
                                skip_group_check=True,
                            )
                    for u in range(2):
                        pr = 2 * j + u
                        nc.vector.tensor_scalar_mul(
                            mct[:, pr, :], mc_ps[:, u, :], inv_den[:, pr : pr + 1]
                        )
                yield
                # piece 4: Mt[c,o] = sum_d wq[d,c] McT[d,o].
                uid[0] += 1
                mt = st["mt"] = small.tile([128, CC, C], bf16, tag="mt", name=f"mt_{uid[0]}")
                for ch in range(CC):
                    mt_ps = ps_mm.tile([128, C], f32, tag="mm")
                    for pr in range(PR):
                        nc.tensor.matmul(
                            mt_ps[:],
                            wqn[:, pr, ch * 128 : (ch + 1) * 128],
                            mct[:, pr, :],
                            start=(pr == 0),
                            stop=(pr == PR - 1),
                        )
                    nc.vector.tensor_copy(mt[:, ch, :], mt_ps[:])
                yield

            def loader(rep, bi):
                if rep < reps:
                    load_x(rep, bi)
                yield

            def chain(*gens):
                for g in gens:
                    yield from g

            def p3_chunk(bi, lt):
                """One l-chunk of y = Mt^T x + b; bias-add alternates ACT/DVE
                so the psum consumer keeps pace with the PE."""
                st = state[bi]
                xt, mt = st["xt"], st["mt"]
                ostg = ostp.tile([128, 2, 512], bf16, tag="ostg")
                for oc2 in range(2):
                    # b1's psf tiles live in the G banks (idle through
                    # phase 3, same shape/tag): their slot-reuse barrier then
                    # references ancient history instead of b0's phase-3
                    # consumers, removing the inter-batch seam stall.
                    if bi == 1:
                        psf = ps_g.tile([128, 512], f32, tag="g")
                    else:
                        psf = ps_mm.tile([128, 512], f32, tag="mm")
                    for cc in range(CC):
                        nc.tensor.matmul(
                            psf[:],
                            mt[:, cc, oc2 * 128 : (oc2 + 1) * 128],
                            xt[:, cc, lt * 512 : (lt + 1) * 512],
                            start=(cc == 0),
                            stop=(cc == CC - 1),
                        )
                    if oc2 == 0:
                        nc.scalar.activation(
                            ostg[:, oc2, :],
                            psf[:],
                            Ident,
                            bias=bb[:, oc2 : oc2 + 1],
                        )
                    else:
                        nc.vector.tensor_scalar_add(
                            ostg[:, oc2, :], psf[:], bb[:, oc2 : oc2 + 1]
                        )
                    nc.sync.dma_start(
                        out_d[bi, oc2, :, lt * 512 : (lt + 1) * 512],
                        ostg[:, oc2, :],
                    )

            def phase3_both(inject):
                """Both batches' output projections; b0's half carries the
                fold(b1) injection between chunks."""
                for lt in range(LT):
                    next(inject, None)
                    p3_chunk(0, lt)
                for lt in range(LT):
                    p3_chunk(1, lt)

            def mk_mid(rep, bi):
                def mid():
                    if rep < reps:
                        load_x(rep, bi)
                return mid

            for rep in range(reps):
                phase1(rep, 0, iter(()))
                phase1(rep, 1, fold(0, mid=mk_mid(rep + 1, 0)))
                phase3_both(fold(1, mid=mk_mid(rep + 1, 1)))

    nc.compile()
    return nc


def _get_nc():
    if "nc" not in _CACHE:
        _CACHE["nc"] = _build()
    return _CACHE["nc"]


def _prep_in_maps(x, w_qkv, w_out, b_out):
    import ml_dtypes

    bf16 = ml_dtypes.bfloat16
    wq_n = np.ascontiguousarray(w_qkv[0:512].reshape(PR, 128, C)).astype(bf16)
    wk_t = np.ascontiguousarray(w_qkv[512:1024].T).reshape(CC, 128, HID).astype(bf16)
    wv_t = np.ascontiguousarray(w_qkv[1024:1536].T).reshape(CC, 128, HID).astype(bf16)
    wo_h = np.ascontiguousarray(
        w_out.T.reshape(HEADS, 64, C).transpose(1, 0, 2)
    ).astype(bf16)
    bb = np.ascontiguousarray(b_out.reshape(2, 128).T).astype(np.float32)
    in_maps = []
    for c in range(N_CORES):
        xs = x[c * NB : (c + 1) * NB].reshape(NB, CC, 128, L).astype(bf16)
        in_maps.append(
            {
                "x": np.ascontiguousarray(xs),
                "wq_n": wq_n,
                "wk_t": wk_t,
                "wv_t": wv_t,
                "wo_h": wo_h,
                "bb": bb,
            }
        )
    return in_maps


def kernel(x, w_qkv, w_out, b_out):
    from concourse.bass_utils import run_bass_kernel_spmd

    nc = _get_nc()
    in_maps = _prep_in_maps(
        np.asarray(x, dtype=np.float32),
        np.asarray(w_qkv, dtype=np.float32),
        np.asarray(w_out, dtype=np.float32),
        np.asarray(b_out, dtype=np.float32),
    )
    res = run_bass_kernel_spmd(nc, in_maps, core_ids=list(range(N_CORES)))
    out = np.concatenate(
        [
            res.results[c]["out"].astype(np.float32).reshape(NB, C, L)
            for c in range(N_CORES)
        ],
        axis=0,
    )
    return out.astype(np.float32)
